# revision 1
# baseline (speedup 1.0000x reference)
"""Trainium2 Bass kernel for nn_ContrastiveLoss (SimCLR-style NT-Xent loss).

Reference computation:
    f = normalize(concat([z1, z2]))            # [2B, D] unit rows
    S = f @ f.T / T                            # [8192, 8192]
    loss = mean_i( logsumexp_j(S[i, :]) - S[i, pos_i] )

Sharding: each of the 8 cores owns a 1024-row block of S and computes it
against all 8192 columns. To keep one SPMD program for all cores, each
core receives the 8 row-groups of F = [z1; z2] ROTATED so its own rows
are always group 0 (the loss is invariant to the column permutation, and
rolling by a multiple of B keeps pos offsets at +4096).

Device-side plan per core:
  1. Cast-load each group's 1024 rows as [128, 8, 512] bf16 (gpsimd DMA
     cast), per-row sum-of-squares (DVE affine_mul_reduce), Quake-style
     Newton rsqrt on DVE, then scale rows to unit norm with the output
     cast straight to fp8e4 in a [p][kh][row-tile][256] layout.
  2. Transpose each (group, k-half) with ONE SBUF->SBUF DMA-xbar
     instruction, treating adjacent fp8 pairs as uint16 units. The
     result [128, 1024] lands exactly in the [128dp, 2pair, cols] layout
     the fp8 DoubleRow matmul wants (contraction d = 256*kh + 2*dp + pair
     consistently on both operands).
  3. fp8e4 DoubleRow matmuls (contraction 256/instr, 2 elem/cycle) build
     [128, 2048] row-blocks of cos-sim in PSUM; ACT computes exp(x/T)
     in place with a fused row-sum (accum_out).
  4. The diagonal (group 0) and positive-pair (group 4) cosines are
     pulled off PSUM pre-exp with an identity-mask mul-reduce on DVE --
     no separate zme/zpos loads.

Outputs per core: "sums" [128, 32] (exp-sums per row over 2048-col
chunks), "diag"/"pos" [128, 8] (raw fp8-precision cosines). Host (f64):
  R_i  = sum(sums_i) - exp(diag_i/T) + exp(1/T)    # exact-diagonal fix
  loss = mean(log(R_i) - pos_i/T)
The diagonal substitution is exact (unit rows have cos=1 analytically)
and cancels the fp8 quantization noise of the dominant softmax term.

The max-subtraction in the reference logsumexp is skipped on device:
|cos|/T <= ~14.3, so sum_j exp() <= ~2e10, well within fp32 range.
"""

import os
import sys

try:
    import concourse.bass  # noqa: F401
except ImportError:
    for _p in ("/root/.axon_site/_ro/trn_rl_repo", "/opt/trn_rl_repo"):
        if _p not in sys.path and os.path.isdir(_p):
            sys.path.insert(0, _p)

import numpy as np

B = 4096
D = 512
T = 0.07
P = 128
NCORES = 8
R = (2 * B) // NCORES  # 1024 rows per core / per group
G = (2 * B) // R       # 8 column groups
GT = R // P            # 8 row tiles per group
H = 2                  # contraction halves (256 each) for DoubleRow
PSW = 2048             # psum tile width (4 banks)
NB = (2 * B) // PSW    # 4 psum tiles per row tile
NS = PSW // 512        # 4 matmul n-slices per psum tile

_NC = None


def _build():
    from contextlib import ExitStack

    import concourse.bacc as bacc
    import concourse.tile as tile
    from concourse import mybir

    f32 = mybir.dt.float32
    bf16 = mybir.dt.bfloat16
    f8 = mybir.dt.float8e4
    u16 = mybir.dt.uint16
    i32 = mybir.dt.int32
    AFT = mybir.ActivationFunctionType
    EXPF = AFT.Exp
    MUL = mybir.AluOpType.mult
    ADD = mybir.AluOpType.add
    SUB = mybir.AluOpType.subtract
    SHR = mybir.AluOpType.logical_shift_right
    DR = mybir.MatmulPerfMode.DoubleRow

    nc = bacc.Bacc(
        "TRN2", target_bir_lowering=False, debug=False, num_devices=NCORES
    )
    fg = [
        nc.dram_tensor(f"f{k}", [R, D], f32, kind="ExternalInput")
        for k in range(G)
    ]
    eye = nc.dram_tensor("eye", [P, P], f32, kind="ExternalInput")
    sums_out = nc.dram_tensor("sums", [P, GT * NB], f32, kind="ExternalOutput")
    diag_out = nc.dram_tensor("diag", [P, GT], f32, kind="ExternalOutput")
    pos_out = nc.dram_tensor("pos", [P, GT], f32, kind="ExternalOutput")

    with ExitStack() as ctx:
        tc = ctx.enter_context(tile.TileContext(nc))
        smalls = ctx.enter_context(tc.tile_pool(name="smalls", bufs=1))
        dumps = ctx.enter_context(tc.tile_pool(name="dumps", bufs=4))
        stats = ctx.enter_context(tc.tile_pool(name="stats", bufs=3))
        zbpool = ctx.enter_context(tc.tile_pool(name="zbpool", bufs=3))
        fnpool = ctx.enter_context(tc.tile_pool(name="fnpool", bufs=2))
        ftpool = ctx.enter_context(tc.tile_pool(name="ftpool", bufs=1))
        psum = ctx.enter_context(tc.tile_pool(name="psum", bufs=2, space="PSUM"))

        sums_sb = smalls.tile([P, GT * NB], f32, tag="sums_sb")
        diag_sb = smalls.tile([P, GT], f32, tag="diag_sb")
        pos_sb = smalls.tile([P, GT], f32, tag="pos_sb")
        eye_sb = smalls.tile([P, P], f32, tag="eye_sb")
        nc.sync.dma_start(out=eye_sb[:], in_=eye[:, :])
        magic = smalls.tile([P, GT], i32, tag="magic")
        nc.vector.memset(magic[:], 0x5F3759DF)

        def mulsum(in0, in1, accum_col):
            # accum_col[p] = sum_x in0[p,x]*in1[p,x] in one DVE op; the
            # mandatory main output goes to a throwaway broadcast AP.
            dummy = dumps.tile([P, 1], f32, tag="dummy")
            nc.vector.affine_mul_reduce(
                out=dummy.broadcast_to(in0.shape),
                accum_out=accum_col,
                in0=in0,
                in1=in1,
                scale=1.0,
                bias=0.0,
            )

        def rsqrt(invn_dst, ssq):
            # 1/max(sqrt(s), eps) == min(rsqrt(s), 1e12); Quake bit-trick
            # + 2 Newton iterations, all on DVE -- keeps ACT's table set
            # pinned to Exp for the whole kernel.
            n = ssq.shape[1]
            h = stats.tile([P, n], i32, tag="h")
            nc.vector.tensor_scalar(h[:], ssq.bitcast(i32), 1, None, op0=SHR)
            y = stats.tile([P, n], f32, tag="y")
            nc.vector.tensor_tensor(y[:].bitcast(i32), magic[:, :n], h[:], op=SUB)
            a = stats.tile([P, n], f32, tag="a")
            for _ in range(2):
                nc.vector.tensor_mul(a[:], y[:], y[:])
                nc.vector.tensor_mul(a[:], a[:], ssq)
                nc.vector.tensor_scalar(a[:], a[:], -0.5, 1.5, op0=MUL, op1=ADD)
                nc.vector.tensor_mul(y[:], y[:], a[:])
            nc.vector.tensor_scalar_min(invn_dst, y[:], 1.0e12)

        ft2 = [[None] * H for _ in range(G)]
        # Deinterleaved operand tiles [dp][h][pair][j] with pair stride R:
        # the DoubleRow LDWEIGHTS ISA check requires pair stride %16, and
        # the byte-interleaved ft2 layout also defeats the double-pumped
        # ifmap stream (measured 1 elem/cycle); production kernels use
        # far-strided k-pairs with contiguous columns. The deinterleave
        # copies run on GpSimd to keep them off the saturated DVE.
        wd = [None] * G
        load_insts = []

        def build_group(g):
            from concourse.tile import add_dep_helper

            zb = zbpool.tile([P, GT, D], bf16, tag="zb")
            for s in range(2):
                ld = nc.gpsimd.dma_start(
                    out=zb[:, s * 4 : (s + 1) * 4, :],
                    in_=fg[g][s * 4 * P : (s + 1) * 4 * P, :].rearrange(
                        "(a p) d -> p a d", p=P
                    ),
                )
                # Pace the input stream: all 16 loads are dep-free, and
                # unpaced they round-robin across DMA queues so group 0
                # finishes no earlier than group 7 (PE idles ~40us).
                # Chaining g behind g-2 keeps ~2 groups in flight.
                if g >= 2:
                    add_dep_helper(
                        ld.ins, load_insts[2 * (g - 2) + s].ins, reason="pace loads"
                    )
                load_insts.append(ld)
            ssq = stats.tile([P, GT], f32, tag="ssq")
            for a in range(GT):
                mulsum(zb[:, a, :], zb[:, a, :], ssq[:, a : a + 1])
            invn = stats.tile([P, GT], f32, tag="invn")
            rsqrt(invn[:], ssq[:])
            # fn8 layout: [p][kh][row-tile][256] so each k-half is a
            # contiguous [128, 2048] fp8 = [128, 1024] u16 transpose src.
            fn8 = fnpool.tile([P, H, GT, D // H], f8, tag="fn8")
            for a in range(GT):
                nc.vector.tensor_scalar_mul(
                    fn8[:, :, a, :],
                    zb[:, a, :].rearrange("p (h x) -> p h x", h=H),
                    invn[:, a : a + 1],
                )
            for h in range(H):
                fth = ftpool.tile([P, R], u16, tag=f"ft{g}_{h}", name=f"ft{g}_{h}")
                nc.sync.dma_start(
                    out=fth[:].rearrange("p (a j) -> p a j", a=GT),
                    in_=fn8[:, h].bitcast(u16),
                    transpose=True,
                )
                ft2[g][h] = fth
            # Only own rows (the stationary operand) need the deinterleaved
            # layout. Deinterleaving the rhs groups too was measured SLOWER
            # overall on both GpSimd (290us; 7us/copy gated the matmuls)
            # and DVE (222us; +24us on the critical DVE engine) despite
            # faster matmuls, so the moving operand reads ft2 directly.
            if g == 0:
                wdg = ftpool.tile([P, H, 2, R], f8, tag="wd0", name="wd0")
                for h in range(H):
                    nc.vector.tensor_copy(wdg[:, h], f8view(0, h))
                wd[0] = wdg

        def f8view(g, h):
            # [128, 1024] u16 -> [128 dp, 2 pair, 1024 cols] fp8; the
            # contraction index is d = 256*h + 2*dp + pair on BOTH sides.
            return ft2[g][h][:].bitcast(f8).rearrange(
                "p (j two) -> p two j", two=2
            )

        def sim_block(nb):
            for r in range(GT):
                ps = psum.tile([P, PSW], f32, tag="ps")
                for h in range(H):
                    lhsT = wd[0][:, h, :, r * P : (r + 1) * P]
                    for ns in range(NS):
                        j0 = nb * PSW + ns * 512
                        gj, cj = divmod(j0, R)
                        nc.tensor.matmul(
                            ps[:, ns * 512 : (ns + 1) * 512],
                            lhsT,
                            f8view(gj, h)[:, :, cj : cj + 512],
                            start=(h == 0),
                            stop=(h == H - 1),
                            perf_mode=DR,
                        )
                # Raw-cosine extraction must read PSUM before the in-place
                # exp. Own rows are group 0, pos pairs group 4 (rolled).
                if nb == 0:
                    mulsum(ps[:, r * P : (r + 1) * P], eye_sb[:], diag_sb[:, r : r + 1])
                if nb == 2:
                    mulsum(ps[:, r * P : (r + 1) * P], eye_sb[:], pos_sb[:, r : r + 1])
                idx = r * NB + nb
                nc.scalar.activation(
                    ps[:],
                    ps[:],
                    EXPF,
                    scale=1.0 / T,
                    accum_out=sums_sb[:, idx : idx + 1],
                )

        # Program order doubles as scheduler priority: the two groups a
        # column-block needs are built right before its matmuls; later
        # groups' loads gap-fill under PE/ACT work.
        for nb in range(NB):
            build_group(2 * nb)
            build_group(2 * nb + 1)
            sim_block(nb)

        nc.sync.dma_start(out=sums_out[:], in_=sums_sb[:])
        nc.sync.dma_start(out=diag_out[:], in_=diag_sb[:])
        nc.sync.dma_start(out=pos_out[:], in_=pos_sb[:])

    nc.compile()
    return nc


def _get_nc():
    global _NC
    if _NC is None:
        _NC = _build()
    return _NC


def run(z1, z2, trace=False):
    """Run the SPMD kernel; returns (loss, BassKernelResults)."""
    from concourse.bass_utils import run_bass_kernel_spmd

    z1 = np.ascontiguousarray(z1, dtype=np.float32)
    z2 = np.ascontiguousarray(z2, dtype=np.float32)
    F = np.concatenate([z1, z2], axis=0)  # [8192, 512]
    eye_np = np.eye(P, dtype=np.float32)
    in_maps = []
    for c in range(NCORES):
        m = {"eye": eye_np}
        for k in range(G):
            blk = (c + k) % G
            m[f"f{k}"] = F[blk * R : (blk + 1) * R]
        in_maps.append(m)
    res = run_bass_kernel_spmd(
        _get_nc(), in_maps, core_ids=list(range(NCORES)), trace=trace
    )
    e_diag_true = np.exp(1.0 / T)
    total = 0.0
    for r in res.results:
        sums = r["sums"].astype(np.float64)  # [P, GT*NB]
        diag = r["diag"].astype(np.float64)  # [P, GT] own-cos (~1 + fp8 noise)
        pos = r["pos"].astype(np.float64)    # [P, GT] positive-pair cosines
        sumexp = sums.reshape(P, GT, NB).sum(axis=2)
        sumexp = sumexp - np.exp(diag / T) + e_diag_true
        total += (np.log(sumexp) - pos / T).sum()
    loss = total / (2.0 * B)
    return np.float32(loss), res


def kernel(z1, z2, labels=None, **_ignored):
    loss, _ = run(z1, z2, trace=False)
    return np.asarray(loss, dtype=np.float32)


if __name__ == "__main__":
    rng = np.random.default_rng(0)
    a = rng.standard_normal((B, D)).astype(np.float32)
    b = rng.standard_normal((B, D)).astype(np.float32)
    print(kernel(a, b, None))



# revision 11
# speedup vs baseline: 1.2657x; 1.2657x over previous
"""Trainium2 Bass kernel for nn_ContrastiveLoss (SimCLR-style NT-Xent loss).

Reference computation:
    f = normalize(concat([z1, z2]))            # [2B, D] unit rows
    S = f @ f.T / T                            # [8192, 8192]
    loss = mean_i( logsumexp_j(S[i, :]) - S[i, pos_i] )

Symmetric sharding: S is symmetric, so each core computes only 5 of the
8 column-groups of its 1024-row block (groups 0..4 after rotating the 8
row-groups so the core's own rows are group 0).  The missing column
groups 5,6,7 of row-block b are the transposes of blocks computed by
cores b-3, b-2, b-1, and are recovered as COLUMN sums of the exp'd
blocks g=1..3 (a tiny fp8 DoubleRow ones-matmul per block), exchanged
between cores on the host during the final (cheap) reduction.  This cuts
matmul + exp work to 5/8 and HBM traffic to 10 MB/core.

Operand layout: rows are normalized in row-major bf16 (DVE sum-of-
squares + Quake rsqrt + scale), DMA-xbar transposed as native 2-byte
elements into [dp, db, col] (d = 128*db + dp), then cast to fp8e4.  A
DoubleRow matmul contraction pair (dp, t) maps to d = 256h + 128t + dp,
so BOTH operands slice straight out of the same [128, 4, 1024] fp8 tile
with far-strided (1024B) k-pairs and contiguous columns - the layout the
double-pumped weight/ifmap streams require (byte-interleaved pairs run
at 1 elem/cycle).  NOTE: tensor_tensor_reduce hangs TRN2 hardware (sim
is fine) - all mul-reduces must use affine_mul_reduce.

Per 128-row tile r and group g, the [128, 1024] psum block gets:
  g=0: diag extraction (eye mul-reduce, pre-exp), in-place exp with
       fused row-sum (accum_out).
  g=4: pos-pair extraction (same trick; pos offsets are +4B rows = group
       4 after rotation), in-place exp + row-sum.
  g=1..3: exp written as fp8e4 to SBUF (off-diagonal cosines are within
       +-0.25 whp, so exp(S/T) spans ~[e-4, e4] - inside fp8e4 range)
       + fused row-sum; pairs of row-tiles feed a [128, 2, 512] DR
       ones-matmul accumulating column sums in psum.

Host (f64) assembles denominators across cores:
  den[b] = rowsums_b - exp(diag_b/T) + e^{1/T} + sum_g colsums_{b-g}[g]
  loss   = mean(log(den) - pos/T)
The exact-diagonal substitution cancels the fp8 quantization noise of
the dominant e^{1/T} ~ 1.6e6 softmax term (the rest of a row sums to
~1e4), exactly as in the v1 kernel.  No logsumexp max-subtraction is
needed: sum_j exp() <= ~2e10 fits fp32.
"""

import os
import sys

try:
    import concourse.bass  # noqa: F401
except ImportError:
    for _p in ("/root/.axon_site/_ro/trn_rl_repo", "/opt/trn_rl_repo"):
        if _p not in sys.path and os.path.isdir(_p):
            sys.path.insert(0, _p)

import numpy as np

B = 4096
D = 512
T = 0.07
P = 128
NCORES = 8
R = (2 * B) // NCORES  # 1024 rows per block
G = 8                  # total row/col blocks
NG = 5                 # column groups computed per core (symmetry)
GT = R // P            # 8 row tiles per block
H = 2                  # DR contraction halves (256 each)
DB = D // P            # 4 d-blocks of 128

_NC = None


def _build():
    from contextlib import ExitStack

    import concourse.bacc as bacc
    import concourse.tile as tile
    from concourse import mybir
    from concourse.tile import add_dep_helper

    f32 = mybir.dt.float32
    bf16 = mybir.dt.bfloat16
    f8 = mybir.dt.float8e4
    i32 = mybir.dt.int32
    AFT = mybir.ActivationFunctionType
    EXPF = AFT.Exp
    MUL = mybir.AluOpType.mult
    ADD = mybir.AluOpType.add
    SUB = mybir.AluOpType.subtract
    SHR = mybir.AluOpType.logical_shift_right
    DR = mybir.MatmulPerfMode.DoubleRow

    nc = bacc.Bacc(
        "TRN2", target_bir_lowering=False, debug=False, num_devices=NCORES
    )
    fg = [
        nc.dram_tensor(f"f{k}", [R, D], f32, kind="ExternalInput")
        for k in range(NG)
    ]
    eye = nc.dram_tensor("eye", [P, P], f32, kind="ExternalInput")
    sums_out = nc.dram_tensor("sums", [P, NG * GT], f32, kind="ExternalOutput")
    diag_out = nc.dram_tensor("diag", [P, GT], f32, kind="ExternalOutput")
    pos_out = nc.dram_tensor("pos", [P, GT], f32, kind="ExternalOutput")
    csum_out = nc.dram_tensor("csum", [1, 3 * R], f32, kind="ExternalOutput")

    with ExitStack() as ctx:
        tc = ctx.enter_context(tile.TileContext(nc))
        smalls = ctx.enter_context(tc.tile_pool(name="smalls", bufs=1))
        dumps = ctx.enter_context(tc.tile_pool(name="dumps", bufs=4))
        stats = ctx.enter_context(tc.tile_pool(name="stats", bufs=3))
        zbpool = ctx.enter_context(tc.tile_pool(name="zbpool", bufs=2))
        tbpool = ctx.enter_context(tc.tile_pool(name="tbpool", bufs=2))
        f8pool = ctx.enter_context(tc.tile_pool(name="f8pool", bufs=1))
        e8pool = ctx.enter_context(tc.tile_pool(name="e8pool", bufs=2))
        psum = ctx.enter_context(tc.tile_pool(name="psum", bufs=3, space="PSUM"))
        cspool = ctx.enter_context(tc.tile_pool(name="cspool", bufs=1, space="PSUM"))

        sums_sb = smalls.tile([P, NG * GT], f32, tag="sums_sb")
        diag_sb = smalls.tile([P, GT], f32, tag="diag_sb")
        pos_sb = smalls.tile([P, GT], f32, tag="pos_sb")
        csum_sb = smalls.tile([1, 3 * R], f32, tag="csum_sb")
        eye_sb = smalls.tile([P, P], f32, tag="eye_sb")
        nc.sync.dma_start(out=eye_sb[:], in_=eye[:, :])
        magic = smalls.tile([P, GT], i32, tag="magic")
        nc.vector.memset(magic[:], 0x5F3759DF)
        # DR stationary all-ones [128, 2, 1] with 16B pair stride.
        ones8 = smalls.tile([P, 2, 16], f8, tag="ones8")
        nc.vector.memset(ones8[:], 1.0)

        def mulsum(in0, in1, accum_col):
            # accum_col[p] = sum_x in0[p,x]*in1[p,x]; main out is a
            # throwaway broadcast AP.  (tensor_tensor_reduce would do the
            # same in one standard op but hangs TRN2 hardware.)
            dummy = dumps.tile([P, 1], f32, tag="dummy")
            nc.vector.affine_mul_reduce(
                out=dummy.broadcast_to(in0.shape),
                accum_out=accum_col,
                in0=in0,
                in1=in1,
                scale=1.0,
                bias=0.0,
            )

        def rsqrt(invn_dst, ssq):
            # 1/max(sqrt(s), eps) == min(rsqrt(s), 1e12); Quake bit-trick
            # + 2 Newton iterations, all on DVE.
            n = ssq.shape[1]
            h = stats.tile([P, n], i32, tag="h")
            nc.vector.tensor_scalar(h[:], ssq.bitcast(i32), 1, None, op0=SHR)
            y = stats.tile([P, n], f32, tag="y")
            nc.vector.tensor_tensor(y[:].bitcast(i32), magic[:, :n], h[:], op=SUB)
            a = stats.tile([P, n], f32, tag="a")
            for _ in range(2):
                nc.vector.tensor_mul(a[:], y[:], y[:])
                nc.vector.tensor_mul(a[:], a[:], ssq)
                nc.vector.tensor_scalar(a[:], a[:], -0.5, 1.5, op0=MUL, op1=ADD)
                nc.vector.tensor_mul(y[:], y[:], a[:])
            nc.vector.tensor_scalar_min(invn_dst, y[:], 1.0e12)

        load_insts = []
        zbs = {}

        def load_group(g):
            zb = zbpool.tile([P, GT, D], bf16, tag="zb")
            for s in range(2):
                ld = nc.gpsimd.dma_start(
                    out=zb[:, s * 4 : (s + 1) * 4, :],
                    in_=fg[g][s * 4 * P : (s + 1) * 4 * P, :].rearrange(
                        "(a p) d -> p a d", p=P
                    ),
                )
                # Serial-chain the loads: group 0 must land first (it is
                # both the stationary operand and the first rhs); unpaced
                # loads round-robin across SDMA queues and all finish
                # together.
                if load_insts:
                    add_dep_helper(ld.ins, load_insts[-1].ins, reason="pace loads")
                load_insts.append(ld)
            zbs[g] = zb

        ft8s = {}

        def prep_group(g, cast_engine):
            zb = zbs.pop(g)
            ssq = stats.tile([P, GT], f32, tag="ssq")
            for a in range(GT):
                mulsum(zb[:, a, :], zb[:, a, :], ssq[:, a : a + 1])
            invn = stats.tile([P, GT], f32, tag="invn")
            rsqrt(invn[:], ssq[:])
            for a in range(GT):
                nc.vector.tensor_scalar_mul(
                    zb[:, a, :], zb[:, a, :], invn[:, a : a + 1]
                )
            # Native 2-byte xbar transposes (one per row-tile a, keeping
            # both APs within the 2D-in/3D-out transpose constraint):
            # tb[dp, db, a*128+j] = zb[j, a, 128*db+dp], i.e. F^T with
            # d = 128*db + dp on the partition axis and columns ordered
            # like rows (a*128+j).
            tb = tbpool.tile([P, DB, R], bf16, tag="tb")
            for a in range(GT):
                nc.sync.dma_start(
                    out=tb[:, :, a * P : (a + 1) * P],
                    in_=zb[:, a, :],
                    transpose=True,
                )
            ft8 = f8pool.tile([P, DB, R], f8, tag=f"ft8_{g}", name=f"ft8_{g}")
            cast_engine.tensor_copy(ft8[:], tb[:])
            ft8s[g] = ft8

        def sim_phase(g):
            ft8g = ft8s[g]
            ft80 = ft8s[0]
            cs = None
            if g in (1, 2, 3):
                cs = cspool.tile([P, R], f32, tag="cs")
            e8 = None
            for r in range(GT):
                ps = psum.tile([P, R], f32, tag="ps")
                for h in range(H):
                    lhsT = ft80[:, 2 * h : 2 * h + 2, r * P : (r + 1) * P]
                    for ns in range(2):
                        nc.tensor.matmul(
                            ps[:, ns * 512 : (ns + 1) * 512],
                            lhsT,
                            ft8g[:, 2 * h : 2 * h + 2, ns * 512 : (ns + 1) * 512],
                            start=(h == 0),
                            stop=(h == H - 1),
                            perf_mode=DR,
                        )
                # Raw-cosine extraction reads PSUM before/parallel to exp.
                if g == 0:
                    mulsum(ps[:, r * P : (r + 1) * P], eye_sb[:], diag_sb[:, r : r + 1])
                if g == 4:
                    mulsum(ps[:, r * P : (r + 1) * P], eye_sb[:], pos_sb[:, r : r + 1])
                acc = sums_sb[:, g * GT + r : g * GT + r + 1]
                if g in (1, 2, 3):
                    if r % 2 == 0:
                        e8 = e8pool.tile([P, 2, R], f8, tag="e8")
                    nc.scalar.activation(
                        e8[:, r % 2, :], ps[:], EXPF, scale=1.0 / T, accum_out=acc
                    )
                    if r % 2 == 1:
                        pr = r // 2
                        for ns in range(2):
                            nc.tensor.matmul(
                                cs[0:1, ns * 512 : (ns + 1) * 512],
                                ones8[:, :, 0:1],
                                e8[:, :, ns * 512 : (ns + 1) * 512],
                                start=(pr == 0),
                                stop=(pr == GT // 2 - 1),
                                perf_mode=DR,
                            )
                else:
                    nc.scalar.activation(
                        ps[:], ps[:], EXPF, scale=1.0 / T, accum_out=acc
                    )
            if g in (1, 2, 3):
                nc.vector.tensor_copy(
                    csum_sb[0:1, (g - 1) * R : g * R], cs[0:1, :]
                )

        # Startup: groups 0 and 1 loaded+prepped before phase 0; later
        # groups stream in two phases ahead of use.
        load_group(0)
        load_group(1)
        prep_group(0, nc.vector)
        prep_group(1, nc.vector)
        for g in range(NG):
            if g + 2 < NG:
                load_group(g + 2)
            sim_phase(g)
            if g + 2 < NG:
                prep_group(g + 2, nc.vector)

        nc.sync.dma_start(out=sums_out[:], in_=sums_sb[:])
        nc.sync.dma_start(out=diag_out[:], in_=diag_sb[:])
        nc.sync.dma_start(out=pos_out[:], in_=pos_sb[:])
        nc.sync.dma_start(out=csum_out[:, :], in_=csum_sb[0:1, :])

    nc.compile()
    return nc


def _get_nc():
    global _NC
    if _NC is None:
        _NC = _build()
    return _NC


def run(z1, z2, trace=False):
    """Run the SPMD kernel; returns (loss, BassKernelResults)."""
    from concourse.bass_utils import run_bass_kernel_spmd

    z1 = np.ascontiguousarray(z1, dtype=np.float32)
    z2 = np.ascontiguousarray(z2, dtype=np.float32)
    F = np.concatenate([z1, z2], axis=0)  # [8192, 512]
    eye_np = np.eye(P, dtype=np.float32)
    in_maps = []
    for c in range(NCORES):
        m = {"eye": eye_np}
        for k in range(NG):
            blk = (c + k) % G
            m[f"f{k}"] = F[blk * R : (blk + 1) * R]
        in_maps.append(m)
    res = run_bass_kernel_spmd(
        _get_nc(), in_maps, core_ids=list(range(NCORES)), trace=trace
    )
    e_diag_true = np.exp(1.0 / T)
    # Per-core row-major [1024] views; row i = rt*128 + p.
    RS, DG, PS, CSa = [], [], [], []
    for r in res.results:
        sums = r["sums"].astype(np.float64)  # [P, NG*GT]
        RS.append(sums.reshape(P, NG, GT).sum(axis=1).T.reshape(R))
        DG.append(r["diag"].astype(np.float64).T.reshape(R))
        PS.append(r["pos"].astype(np.float64).T.reshape(R))
        CSa.append(r["csum"].astype(np.float64).reshape(3, R))  # row g-1
    total = 0.0
    for b in range(G):
        den = RS[b] - np.exp(DG[b] / T) + e_diag_true
        for g in (1, 2, 3):
            den = den + CSa[(b - g) % G][g - 1]
        total += (np.log(den) - PS[b] / T).sum()
    loss = total / (2.0 * B)
    return np.float32(loss), res


def kernel(z1, z2, labels=None, **_ignored):
    loss, _ = run(z1, z2, trace=False)
    return np.asarray(loss, dtype=np.float32)


if __name__ == "__main__":
    rng = np.random.default_rng(0)
    a = rng.standard_normal((B, D)).astype(np.float32)
    b = rng.standard_normal((B, D)).astype(np.float32)
    print(kernel(a, b, None))


# revision 14
# speedup vs baseline: 1.4370x; 1.1353x over previous
"""Trainium2 Bass kernel for nn_ContrastiveLoss (SimCLR-style NT-Xent loss).

Reference computation:
    f = normalize(concat([z1, z2]))            # [2B, D] unit rows
    S = f @ f.T / T                            # [8192, 8192]
    loss = mean_i( logsumexp_j(S[i, :]) - S[i, pos_i] )

Symmetric sharding: S is symmetric, so each core computes only 5 of the
8 column-groups of its 1024-row block (groups 0..4 after rotating the 8
row-groups so the core's own rows are group 0).  The missing column
groups 5,6,7 of row-block b are the transposes of blocks computed by
cores b-3, b-2, b-1, and are recovered as COLUMN sums of the exp'd
blocks g=1..3 (a tiny fp8 DoubleRow ones-matmul per block), exchanged
between cores on the host during the final (cheap) reduction.  This cuts
matmul + exp work to 5/8 and HBM traffic to 10 MB/core.

Operand layout: rows are normalized in row-major bf16 (DVE sum-of-
squares + Quake rsqrt + scale), DMA-xbar transposed as native 2-byte
elements into [dp, db, col] (d = 128*db + dp), then cast to fp8e4.  A
DoubleRow matmul contraction pair (dp, t) maps to d = 256h + 128t + dp,
so BOTH operands slice straight out of the same [128, 4, 1024] fp8 tile
with far-strided (1024B) k-pairs and contiguous columns - the layout the
double-pumped weight/ifmap streams require (byte-interleaved pairs run
at 1 elem/cycle).  NOTE: tensor_tensor_reduce hangs TRN2 hardware (sim
is fine) - all mul-reduces must use affine_mul_reduce.

Per 128-row tile r and group g, the [128, 1024] psum block gets:
  g=0: diag extraction (eye mul-reduce, pre-exp), in-place exp with
       fused row-sum (accum_out).
  g=4: pos-pair extraction (same trick; pos offsets are +4B rows = group
       4 after rotation), in-place exp + row-sum.
  g=1..3: exp written as fp8e4 to SBUF (off-diagonal cosines are within
       +-0.25 whp, so exp(S/T) spans ~[e-4, e4] - inside fp8e4 range)
       + fused row-sum; pairs of row-tiles feed a [128, 2, 512] DR
       ones-matmul accumulating column sums in psum.

Host (f64) assembles denominators across cores:
  den[b] = rowsums_b - exp(diag_b/T) + e^{1/T} + sum_g colsums_{b-g}[g]
  loss   = mean(log(den) - pos/T)
The exact-diagonal substitution cancels the fp8 quantization noise of
the dominant e^{1/T} ~ 1.6e6 softmax term (the rest of a row sums to
~1e4), exactly as in the v1 kernel.  No logsumexp max-subtraction is
needed: sum_j exp() <= ~2e10 fits fp32.
"""

import os
import sys

try:
    import concourse.bass  # noqa: F401
except ImportError:
    for _p in ("/root/.axon_site/_ro/trn_rl_repo", "/opt/trn_rl_repo"):
        if _p not in sys.path and os.path.isdir(_p):
            sys.path.insert(0, _p)

import numpy as np

B = 4096
D = 512
T = 0.07
P = 128
NCORES = 8
R = (2 * B) // NCORES  # 1024 rows per block
G = 8                  # total row/col blocks
NG = 5                 # column groups computed per core (symmetry)
GT = R // P            # 8 row tiles per block
H = 2                  # DR contraction halves (256 each)
DB = D // P            # 4 d-blocks of 128

_NC = None


def _build():
    from contextlib import ExitStack

    import concourse.bacc as bacc
    import concourse.tile as tile
    from concourse import mybir
    from concourse.tile import add_dep_helper

    f32 = mybir.dt.float32
    bf16 = mybir.dt.bfloat16
    f8 = mybir.dt.float8e4
    i32 = mybir.dt.int32
    AFT = mybir.ActivationFunctionType
    EXPF = AFT.Exp
    MUL = mybir.AluOpType.mult
    ADD = mybir.AluOpType.add
    SUB = mybir.AluOpType.subtract
    SHR = mybir.AluOpType.logical_shift_right
    DR = mybir.MatmulPerfMode.DoubleRow

    nc = bacc.Bacc(
        "TRN2", target_bir_lowering=False, debug=False, num_devices=NCORES
    )
    fg = [
        nc.dram_tensor(f"f{k}", [R, D], f32, kind="ExternalInput")
        for k in range(NG)
    ]
    eye = nc.dram_tensor("eye", [P, P], f32, kind="ExternalInput")
    sums_out = nc.dram_tensor("sums", [P, NG * GT], f32, kind="ExternalOutput")
    diag_out = nc.dram_tensor("diag", [P, GT], f32, kind="ExternalOutput")
    pos_out = nc.dram_tensor("pos", [P, GT], f32, kind="ExternalOutput")
    csum_out = nc.dram_tensor("csum", [1, 3 * R], f32, kind="ExternalOutput")

    with ExitStack() as ctx:
        tc = ctx.enter_context(tile.TileContext(nc))
        smalls = ctx.enter_context(tc.tile_pool(name="smalls", bufs=1))
        dumps = ctx.enter_context(tc.tile_pool(name="dumps", bufs=4))
        stats = ctx.enter_context(tc.tile_pool(name="stats", bufs=3))
        zbpool = ctx.enter_context(tc.tile_pool(name="zbpool", bufs=2))
        tbpool = ctx.enter_context(tc.tile_pool(name="tbpool", bufs=2))
        f8pool = ctx.enter_context(tc.tile_pool(name="f8pool", bufs=1))
        e8pool = ctx.enter_context(tc.tile_pool(name="e8pool", bufs=2))
        psum = ctx.enter_context(tc.tile_pool(name="psum", bufs=3, space="PSUM"))
        cspool = ctx.enter_context(tc.tile_pool(name="cspool", bufs=1, space="PSUM"))

        sums_sb = smalls.tile([P, NG * GT], f32, tag="sums_sb")
        diag_sb = smalls.tile([P, GT], f32, tag="diag_sb")
        pos_sb = smalls.tile([P, GT], f32, tag="pos_sb")
        csum_sb = smalls.tile([1, 3 * R], f32, tag="csum_sb")
        eye_sb = smalls.tile([P, P], f32, tag="eye_sb")
        nc.sync.dma_start(out=eye_sb[:], in_=eye[:, :])
        magic = smalls.tile([P, GT], i32, tag="magic")
        nc.vector.memset(magic[:], 0x5F3759DF)
        # DR stationary all-ones [128, 2, 1] with 16B pair stride.
        ones8 = smalls.tile([P, 2, 16], f8, tag="ones8")
        nc.vector.memset(ones8[:], 1.0)

        def mulsum(in0, in1, accum_col):
            # accum_col[p] = sum_x in0[p,x]*in1[p,x]; main out is a
            # throwaway broadcast AP.  (tensor_tensor_reduce would do the
            # same in one standard op but hangs TRN2 hardware.)
            dummy = dumps.tile([P, 1], f32, tag="dummy")
            nc.vector.affine_mul_reduce(
                out=dummy.broadcast_to(in0.shape),
                accum_out=accum_col,
                in0=in0,
                in1=in1,
                scale=1.0,
                bias=0.0,
            )

        def rsqrt(invn_dst, ssq):
            # 1/max(sqrt(s), eps) == min(rsqrt(s), 1e12); Quake bit-trick
            # + 2 Newton iterations, all on DVE.
            n = ssq.shape[1]
            h = stats.tile([P, n], i32, tag="h")
            nc.vector.tensor_scalar(h[:], ssq.bitcast(i32), 1, None, op0=SHR)
            y = stats.tile([P, n], f32, tag="y")
            nc.vector.tensor_tensor(y[:].bitcast(i32), magic[:, :n], h[:], op=SUB)
            a = stats.tile([P, n], f32, tag="a")
            for _ in range(2):
                nc.vector.tensor_mul(a[:], y[:], y[:])
                nc.vector.tensor_mul(a[:], a[:], ssq)
                nc.vector.tensor_scalar(a[:], a[:], -0.5, 1.5, op0=MUL, op1=ADD)
                nc.vector.tensor_mul(y[:], y[:], a[:])
            nc.vector.tensor_scalar_min(invn_dst, y[:], 1.0e12)

        load_insts = []
        zbs = {}

        def load_group(g):
            # Pacing: group 0's two chunks serial-chained (lowest latency
            # for the startup-critical stationary operand: one cast-DMA
            # only sustains ~147 GB/s), later chunks 3-wide to approach
            # the ~358 GB/s HBM limit without starving chunk order.
            zb = zbpool.tile([P, GT, D], bf16, tag="zb")
            for s in range(2):
                ld = nc.gpsimd.dma_start(
                    out=zb[:, s * 4 : (s + 1) * 4, :],
                    in_=fg[g][s * 4 * P : (s + 1) * 4 * P, :].rearrange(
                        "(a p) d -> p a d", p=P
                    ),
                )
                n = len(load_insts)
                dep = n - 1 if n <= 2 else (n - 3 if n >= 5 else 1)
                if n >= 1:
                    add_dep_helper(
                        ld.ins, load_insts[dep].ins, reason="pace loads"
                    )
                load_insts.append(ld)
            zbs[g] = zb

        ft8s = {}

        def prep_group(g, two_queues=False):
            zb = zbs.pop(g)
            ssq = stats.tile([P, GT], f32, tag="ssq")
            for a in range(GT):
                mulsum(zb[:, a, :], zb[:, a, :], ssq[:, a : a + 1])
            invn = stats.tile([P, GT], f32, tag="invn")
            rsqrt(invn[:], ssq[:])
            for a in range(GT):
                nc.vector.tensor_scalar_mul(
                    zb[:, a, :], zb[:, a, :], invn[:, a : a + 1]
                )
            # Native 2-byte xbar transposes (one per row-tile a, keeping
            # both APs within the 2D-in/3D-out transpose constraint):
            # tb[dp, db, a*128+j] = zb[j, a, 128*db+dp], i.e. F^T with
            # d = 128*db + dp on the partition axis and columns ordered
            # like rows (a*128+j).  During startup (ACT idle) they are
            # split over both HWDGE queues (SP + ACT) to halve the serial
            # 8 x ~1.27us queue time; in steady state ACT runs exps, so
            # everything stays on the SP queue.
            tb = tbpool.tile([P, DB, R], bf16, tag="tb")
            for a in range(GT):
                q = nc.scalar if (two_queues and a % 2 == 1) else nc.sync
                q.dma_start(
                    out=tb[:, :, a * P : (a + 1) * P],
                    in_=zb[:, a, :],
                    transpose=True,
                )
            ft8 = f8pool.tile([P, DB, R], f8, tag=f"ft8_{g}", name=f"ft8_{g}")
            nc.vector.tensor_copy(ft8[:], tb[:])
            ft8s[g] = ft8

        def sim_phase(g):
            ft8g = ft8s[g]
            ft80 = ft8s[0]
            cs = None
            if g in (1, 2, 3):
                cs = cspool.tile([P, R], f32, tag="cs")
            e8 = None
            for r in range(GT):
                ps = psum.tile([P, R], f32, tag="ps")
                for h in range(H):
                    lhsT = ft80[:, 2 * h : 2 * h + 2, r * P : (r + 1) * P]
                    for ns in range(2):
                        nc.tensor.matmul(
                            ps[:, ns * 512 : (ns + 1) * 512],
                            lhsT,
                            ft8g[:, 2 * h : 2 * h + 2, ns * 512 : (ns + 1) * 512],
                            start=(h == 0),
                            stop=(h == H - 1),
                            perf_mode=DR,
                        )
                # Raw-cosine extraction reads PSUM before/parallel to exp.
                if g == 0:
                    mulsum(ps[:, r * P : (r + 1) * P], eye_sb[:], diag_sb[:, r : r + 1])
                if g == 4:
                    mulsum(ps[:, r * P : (r + 1) * P], eye_sb[:], pos_sb[:, r : r + 1])
                acc = sums_sb[:, g * GT + r : g * GT + r + 1]
                if g in (1, 2, 3):
                    if r % 2 == 0:
                        e8 = e8pool.tile([P, 2, R], f8, tag="e8")
                    nc.scalar.activation(
                        e8[:, r % 2, :], ps[:], EXPF, scale=1.0 / T, accum_out=acc
                    )
                    if r % 2 == 1:
                        pr = r // 2
                        for ns in range(2):
                            nc.tensor.matmul(
                                cs[0:1, ns * 512 : (ns + 1) * 512],
                                ones8[:, :, 0:1],
                                e8[:, :, ns * 512 : (ns + 1) * 512],
                                start=(pr == 0),
                                stop=(pr == GT // 2 - 1),
                                perf_mode=DR,
                            )
                else:
                    nc.scalar.activation(
                        ps[:], ps[:], EXPF, scale=1.0 / T, accum_out=acc
                    )
            if g in (1, 2, 3):
                nc.vector.tensor_copy(
                    csum_sb[0:1, (g - 1) * R : g * R], cs[0:1, :]
                )

        # Startup: groups 0 and 1 loaded+prepped before phase 0; later
        # groups stream in two phases ahead of use.  prep(g+2) is emitted
        # BEFORE sim_phase(g): its DVE ops sit ahead in the queue and run
        # as soon as load(g+2) lands, during phase g's PE/ACT work.
        load_group(0)
        load_group(1)
        prep_group(0, two_queues=True)
        prep_group(1, two_queues=True)
        for g in range(NG):
            if g + 2 < NG:
                load_group(g + 2)
                prep_group(g + 2)
            sim_phase(g)

        nc.sync.dma_start(out=sums_out[:], in_=sums_sb[:])
        nc.sync.dma_start(out=diag_out[:], in_=diag_sb[:])
        nc.sync.dma_start(out=pos_out[:], in_=pos_sb[:])
        nc.sync.dma_start(out=csum_out[:, :], in_=csum_sb[0:1, :])

    nc.compile()
    return nc


def _get_nc():
    global _NC
    if _NC is None:
        _NC = _build()
    return _NC


def run(z1, z2, trace=False):
    """Run the SPMD kernel; returns (loss, BassKernelResults)."""
    from concourse.bass_utils import run_bass_kernel_spmd

    z1 = np.ascontiguousarray(z1, dtype=np.float32)
    z2 = np.ascontiguousarray(z2, dtype=np.float32)
    F = np.concatenate([z1, z2], axis=0)  # [8192, 512]
    eye_np = np.eye(P, dtype=np.float32)
    in_maps = []
    for c in range(NCORES):
        m = {"eye": eye_np}
        for k in range(NG):
            blk = (c + k) % G
            m[f"f{k}"] = F[blk * R : (blk + 1) * R]
        in_maps.append(m)
    res = run_bass_kernel_spmd(
        _get_nc(), in_maps, core_ids=list(range(NCORES)), trace=trace
    )
    e_diag_true = np.exp(1.0 / T)
    # Per-core row-major [1024] views; row i = rt*128 + p.
    RS, DG, PS, CSa = [], [], [], []
    for r in res.results:
        sums = r["sums"].astype(np.float64)  # [P, NG*GT]
        RS.append(sums.reshape(P, NG, GT).sum(axis=1).T.reshape(R))
        DG.append(r["diag"].astype(np.float64).T.reshape(R))
        PS.append(r["pos"].astype(np.float64).T.reshape(R))
        CSa.append(r["csum"].astype(np.float64).reshape(3, R))  # row g-1
    total = 0.0
    for b in range(G):
        den = RS[b] - np.exp(DG[b] / T) + e_diag_true
        for g in (1, 2, 3):
            den = den + CSa[(b - g) % G][g - 1]
        total += (np.log(den) - PS[b] / T).sum()
    loss = total / (2.0 * B)
    return np.float32(loss), res


def kernel(z1, z2, labels=None, **_ignored):
    loss, _ = run(z1, z2, trace=False)
    return np.asarray(loss, dtype=np.float32)


if __name__ == "__main__":
    rng = np.random.default_rng(0)
    a = rng.standard_normal((B, D)).astype(np.float32)
    b = rng.standard_normal((B, D)).astype(np.float32)
    print(kernel(a, b, None))


# revision 19
# speedup vs baseline: 1.4489x; 1.0083x over previous
"""Trainium2 Bass kernel for nn_ContrastiveLoss (SimCLR-style NT-Xent loss).

Reference computation:
    f = normalize(concat([z1, z2]))            # [2B, D] unit rows
    S = f @ f.T / T                            # [8192, 8192]
    loss = mean_i( logsumexp_j(S[i, :]) - S[i, pos_i] )

Symmetric sharding: S is symmetric, so each core computes only 5 of the
8 column-groups of its 1024-row block (groups 0..4 after rotating the 8
row-groups so the core's own rows are group 0).  The missing column
groups 5,6,7 of row-block b are the transposes of blocks computed by
cores b-3, b-2, b-1, and are recovered as COLUMN sums of the exp'd
blocks g=1..3 (a tiny fp8 DoubleRow ones-matmul per block), exchanged
between cores on the host during the final (cheap) reduction.  This cuts
matmul + exp work to 5/8 and HBM traffic to 10 MB/core.

Operand layout: rows are normalized in row-major bf16 (DVE sum-of-
squares + Quake rsqrt + scale), DMA-xbar transposed as native 2-byte
elements into [dp, db, col] (d = 128*db + dp), then cast to fp8e4.  A
DoubleRow matmul contraction pair (dp, t) maps to d = 256h + 128t + dp,
so BOTH operands slice straight out of the same [128, 4, 1024] fp8 tile
with far-strided (1024B) k-pairs and contiguous columns - the layout the
double-pumped weight/ifmap streams require (byte-interleaved pairs run
at 1 elem/cycle).  NOTE: tensor_tensor_reduce hangs TRN2 hardware (sim
is fine) - all mul-reduces must use affine_mul_reduce.

Per 128-row tile r and group g, the [128, 1024] psum block gets:
  g=0: diag extraction (eye mul-reduce, pre-exp), in-place exp with
       fused row-sum (accum_out).
  g=4: pos-pair extraction (same trick; pos offsets are +4B rows = group
       4 after rotation), in-place exp + row-sum.
  g=1..3: exp written as fp8e4 to SBUF (off-diagonal cosines are within
       +-0.25 whp, so exp(S/T) spans ~[e-4, e4] - inside fp8e4 range)
       + fused row-sum; pairs of row-tiles feed a [128, 2, 512] DR
       ones-matmul accumulating column sums in psum.

Host (f64) assembles denominators across cores:
  den[b] = rowsums_b - exp(diag_b/T) + e^{1/T} + sum_g colsums_{b-g}[g]
  loss   = mean(log(den) - pos/T)
The exact-diagonal substitution cancels the fp8 quantization noise of
the dominant e^{1/T} ~ 1.6e6 softmax term (the rest of a row sums to
~1e4), exactly as in the v1 kernel.  No logsumexp max-subtraction is
needed: sum_j exp() <= ~2e10 fits fp32.
"""

import os
import sys

try:
    import concourse.bass  # noqa: F401
except ImportError:
    for _p in ("/root/.axon_site/_ro/trn_rl_repo", "/opt/trn_rl_repo"):
        if _p not in sys.path and os.path.isdir(_p):
            sys.path.insert(0, _p)

import numpy as np

B = 4096
D = 512
T = 0.07
P = 128
NCORES = 8
R = (2 * B) // NCORES  # 1024 rows per block
G = 8                  # total row/col blocks
NG = 5                 # column groups computed per core (symmetry)
GT = R // P            # 8 row tiles per block
H = 2                  # DR contraction halves (256 each)
DB = D // P            # 4 d-blocks of 128

_NC = None


def _build():
    from contextlib import ExitStack

    import concourse.bacc as bacc
    import concourse.tile as tile
    from concourse import mybir
    from concourse.tile import add_dep_helper

    f32 = mybir.dt.float32
    bf16 = mybir.dt.bfloat16
    f8 = mybir.dt.float8e4
    i32 = mybir.dt.int32
    AFT = mybir.ActivationFunctionType
    EXPF = AFT.Exp
    MUL = mybir.AluOpType.mult
    ADD = mybir.AluOpType.add
    SUB = mybir.AluOpType.subtract
    SHR = mybir.AluOpType.logical_shift_right
    DR = mybir.MatmulPerfMode.DoubleRow

    nc = bacc.Bacc(
        "TRN2", target_bir_lowering=False, debug=False, num_devices=NCORES
    )
    fg = [
        nc.dram_tensor(f"f{k}", [R, D], f32, kind="ExternalInput")
        for k in range(NG)
    ]
    eye = nc.dram_tensor("eye", [P, P], f32, kind="ExternalInput")
    sums_out = nc.dram_tensor("sums", [P, NG * GT], f32, kind="ExternalOutput")
    diag_out = nc.dram_tensor("diag", [P, GT], f32, kind="ExternalOutput")
    pos_out = nc.dram_tensor("pos", [P, GT], f32, kind="ExternalOutput")
    csum_out = nc.dram_tensor("csum", [1, 3 * R], f32, kind="ExternalOutput")

    with ExitStack() as ctx:
        tc = ctx.enter_context(tile.TileContext(nc))
        smalls = ctx.enter_context(tc.tile_pool(name="smalls", bufs=1))
        dumps = ctx.enter_context(tc.tile_pool(name="dumps", bufs=4))
        stats = ctx.enter_context(tc.tile_pool(name="stats", bufs=3))
        zbpool = ctx.enter_context(tc.tile_pool(name="zbpool", bufs=2))
        tbpool = ctx.enter_context(tc.tile_pool(name="tbpool", bufs=2))
        f8pool = ctx.enter_context(tc.tile_pool(name="f8pool", bufs=1))
        e8pool = ctx.enter_context(tc.tile_pool(name="e8pool", bufs=2))
        scrpool = ctx.enter_context(tc.tile_pool(name="scrpool", bufs=2))
        psum = ctx.enter_context(tc.tile_pool(name="psum", bufs=3, space="PSUM"))
        cspool = ctx.enter_context(tc.tile_pool(name="cspool", bufs=1, space="PSUM"))

        sums_sb = smalls.tile([P, NG * GT], f32, tag="sums_sb")
        diag_sb = smalls.tile([P, GT], f32, tag="diag_sb")
        pos_sb = smalls.tile([P, GT], f32, tag="pos_sb")
        csum_sb = smalls.tile([1, 3 * R], f32, tag="csum_sb")
        eye_sb = smalls.tile([P, P], f32, tag="eye_sb")
        nc.sync.dma_start(out=eye_sb[:], in_=eye[:, :])
        magic = smalls.tile([P, GT], i32, tag="magic")
        nc.vector.memset(magic[:], 0x5F3759DF)
        # DR stationary all-ones [128, 2, 1] with 16B pair stride.
        ones8 = smalls.tile([P, 2, 16], f8, tag="ones8")
        nc.vector.memset(ones8[:], 1.0)

        def mulsum(in0, in1, accum_col):
            # accum_col[p] = sum_x in0[p,x]*in1[p,x]; main out is a
            # throwaway broadcast AP.  (tensor_tensor_reduce would do the
            # same in one standard op but hangs TRN2 hardware.)
            dummy = dumps.tile([P, 1], f32, tag="dummy")
            nc.vector.affine_mul_reduce(
                out=dummy.broadcast_to(in0.shape),
                accum_out=accum_col,
                in0=in0,
                in1=in1,
                scale=1.0,
                bias=0.0,
            )

        def rsqrt(invn_dst, ssq):
            # 1/max(sqrt(s), eps) == min(rsqrt(s), 1e12); Quake bit-trick
            # + 2 Newton iterations, all on DVE.
            n = ssq.shape[1]
            h = stats.tile([P, n], i32, tag="h")
            nc.vector.tensor_scalar(h[:], ssq.bitcast(i32), 1, None, op0=SHR)
            y = stats.tile([P, n], f32, tag="y")
            nc.vector.tensor_tensor(y[:].bitcast(i32), magic[:, :n], h[:], op=SUB)
            a = stats.tile([P, n], f32, tag="a")
            for _ in range(2):
                nc.vector.tensor_mul(a[:], y[:], y[:])
                nc.vector.tensor_mul(a[:], a[:], ssq)
                nc.vector.tensor_scalar(a[:], a[:], -0.5, 1.5, op0=MUL, op1=ADD)
                nc.vector.tensor_mul(y[:], y[:], a[:])
            nc.vector.tensor_scalar_min(invn_dst, y[:], 1.0e12)

        load_insts = []
        zbs = {}

        def load_group(g):
            # Pacing: one cast-DMA only sustains ~150 GB/s, so run two
            # chunks in parallel (~300 GB/s, near the 358 HBM cap) and
            # chain chunk n behind chunk n-2 to keep arrival order.
            zb = zbpool.tile([P, GT, D], bf16, tag="zb")
            for s in range(2):
                ld = nc.gpsimd.dma_start(
                    out=zb[:, s * 4 : (s + 1) * 4, :],
                    in_=fg[g][s * 4 * P : (s + 1) * 4 * P, :].rearrange(
                        "(a p) d -> p a d", p=P
                    ),
                )
                n = len(load_insts)
                if n >= 2:
                    add_dep_helper(
                        ld.ins, load_insts[n - 2].ins, reason="pace loads"
                    )
                load_insts.append(ld)
            zbs[g] = zb

        ft8s = {}

        ssqs = {}

        def prep_ssq(g, a):
            if g not in ssqs:
                ssqs[g] = stats.tile(
                    [P, GT], f32, tag=f"ssq{g % 2}", name=f"ssq_{g}"
                )
            mulsum(zbs[g][:, a, :], zbs[g][:, a, :], ssqs[g][:, a : a + 1])

        def prep_finish(g, two_queues=False):
            # rsqrt + row scale + transpose + fp8 cast for group g (the
            # 8 ssq mul-reduces were already emitted via prep_ssq).
            zb = zbs.pop(g)
            ssq = ssqs.pop(g)
            invn = stats.tile([P, GT], f32, tag="invn")
            rsqrt(invn[:], ssq[:])
            for a in range(GT):
                nc.vector.tensor_scalar_mul(
                    zb[:, a, :], zb[:, a, :], invn[:, a : a + 1]
                )
            # Native 2-byte xbar transposes (one per row-tile a, keeping
            # both APs within the 2D-in/3D-out transpose constraint):
            # tb[dp, db, a*128+j] = zb[j, a, 128*db+dp], i.e. F^T with
            # d = 128*db + dp on the partition axis and columns ordered
            # like rows (a*128+j).  During startup (ACT idle) they are
            # split over both HWDGE queues (SP + ACT) to halve the serial
            # 8 x ~1.27us queue time; in steady state ACT runs exps, so
            # everything stays on the SP queue.  The fp8 cast runs per
            # column half so the first matmuls only wait on 4 transposes.
            tb = tbpool.tile([P, DB, R], bf16, tag="tb")
            ft8 = f8pool.tile([P, DB, R], f8, tag=f"ft8_{g}", name=f"ft8_{g}")
            for half in range(2):
                for a in range(4 * half, 4 * half + 4):
                    q = nc.scalar if (two_queues and a % 2 == 1) else nc.sync
                    q.dma_start(
                        out=tb[:, :, a * P : (a + 1) * P],
                        in_=zb[:, a, :],
                        transpose=True,
                    )
                sl = slice(half * 512, half * 512 + 512)
                nc.vector.tensor_copy(ft8[:, :, sl], tb[:, :, sl])
            ft8s[g] = ft8

        def prep_group(g, two_queues=False):
            for a in range(GT):
                prep_ssq(g, a)
            prep_finish(g, two_queues)

        def sim_phase(g, prep_g=None):
            # prep_g: group whose ssq mul-reduces are drip-fed one per
            # row-tile into the DVE queue (between this phase's psum
            # extractions), with the rsqrt/scale/transpose/cast tail
            # emitted after the phase - ready one full phase before use.
            ft8g = ft8s[g]
            ft80 = ft8s[0]
            cs = None
            if g in (1, 2, 3):
                cs = cspool.tile([P, R], f32, tag="cs")
            e8 = None
            for r in range(GT):
                ps = psum.tile([P, R], f32, tag="ps")
                for h in range(H):
                    lhsT = ft80[:, 2 * h : 2 * h + 2, r * P : (r + 1) * P]
                    for ns in range(2):
                        nc.tensor.matmul(
                            ps[:, ns * 512 : (ns + 1) * 512],
                            lhsT,
                            ft8g[:, 2 * h : 2 * h + 2, ns * 512 : (ns + 1) * 512],
                            start=(h == 0),
                            stop=(h == H - 1),
                            perf_mode=DR,
                        )
                # Raw-cosine extraction reads PSUM in parallel with exp
                # (the exp writes SBUF scratch, not in-place, so there is
                # no write-after-read ordering between them).
                if g == 0:
                    mulsum(ps[:, r * P : (r + 1) * P], eye_sb[:], diag_sb[:, r : r + 1])
                if g == 4:
                    mulsum(ps[:, r * P : (r + 1) * P], eye_sb[:], pos_sb[:, r : r + 1])
                acc = sums_sb[:, g * GT + r : g * GT + r + 1]
                if g in (1, 2, 3):
                    if r % 2 == 0:
                        e8 = e8pool.tile([P, 2, R], f8, tag="e8")
                    nc.scalar.activation(
                        e8[:, r % 2, :], ps[:], EXPF, scale=1.0 / T, accum_out=acc
                    )
                    if r % 2 == 1:
                        pr = r // 2
                        for ns in range(2):
                            nc.tensor.matmul(
                                cs[0:1, ns * 512 : (ns + 1) * 512],
                                ones8[:, :, 0:1],
                                e8[:, :, ns * 512 : (ns + 1) * 512],
                                start=(pr == 0),
                                stop=(pr == GT // 2 - 1),
                                perf_mode=DR,
                            )
                else:
                    scr = scrpool.tile([P, R], bf16, tag="scr")
                    nc.scalar.activation(
                        scr[:], ps[:], EXPF, scale=1.0 / T, accum_out=acc
                    )
                if prep_g is not None:
                    prep_ssq(prep_g, r)
            if g in (1, 2, 3):
                nc.vector.tensor_copy(
                    csum_sb[0:1, (g - 1) * R : g * R], cs[0:1, :]
                )
            if prep_g is not None:
                prep_finish(prep_g)

        # Startup: groups 0 and 1 loaded+prepped before phase 0 (using
        # both HWDGE queues for the transposes while ACT is idle); later
        # groups' loads start immediately and their DVE prep is drip-fed
        # through the preceding phases.
        load_group(0)
        load_group(1)
        prep_group(0, two_queues=True)
        prep_group(1, two_queues=True)
        for g in range(NG):
            if g + 2 < NG:
                load_group(g + 2)
            sim_phase(g, prep_g=g + 2 if g + 2 < NG else None)

        nc.sync.dma_start(out=sums_out[:], in_=sums_sb[:])
        nc.sync.dma_start(out=diag_out[:], in_=diag_sb[:])
        nc.sync.dma_start(out=pos_out[:], in_=pos_sb[:])
        nc.sync.dma_start(out=csum_out[:, :], in_=csum_sb[0:1, :])

    nc.compile()
    return nc


def _get_nc():
    global _NC
    if _NC is None:
        _NC = _build()
    return _NC


def run(z1, z2, trace=False):
    """Run the SPMD kernel; returns (loss, BassKernelResults)."""
    from concourse.bass_utils import run_bass_kernel_spmd

    z1 = np.ascontiguousarray(z1, dtype=np.float32)
    z2 = np.ascontiguousarray(z2, dtype=np.float32)
    F = np.concatenate([z1, z2], axis=0)  # [8192, 512]
    eye_np = np.eye(P, dtype=np.float32)
    in_maps = []
    for c in range(NCORES):
        m = {"eye": eye_np}
        for k in range(NG):
            blk = (c + k) % G
            m[f"f{k}"] = F[blk * R : (blk + 1) * R]
        in_maps.append(m)
    res = run_bass_kernel_spmd(
        _get_nc(), in_maps, core_ids=list(range(NCORES)), trace=trace
    )
    e_diag_true = np.exp(1.0 / T)
    # Per-core row-major [1024] views; row i = rt*128 + p.
    RS, DG, PS, CSa = [], [], [], []
    for r in res.results:
        sums = r["sums"].astype(np.float64)  # [P, NG*GT]
        RS.append(sums.reshape(P, NG, GT).sum(axis=1).T.reshape(R))
        DG.append(r["diag"].astype(np.float64).T.reshape(R))
        PS.append(r["pos"].astype(np.float64).T.reshape(R))
        CSa.append(r["csum"].astype(np.float64).reshape(3, R))  # row g-1
    total = 0.0
    for b in range(G):
        den = RS[b] - np.exp(DG[b] / T) + e_diag_true
        for g in (1, 2, 3):
            den = den + CSa[(b - g) % G][g - 1]
        total += (np.log(den) - PS[b] / T).sum()
    loss = total / (2.0 * B)
    return np.float32(loss), res


def kernel(z1, z2, labels=None, **_ignored):
    loss, _ = run(z1, z2, trace=False)
    return np.asarray(loss, dtype=np.float32)


if __name__ == "__main__":
    rng = np.random.default_rng(0)
    a = rng.standard_normal((B, D)).astype(np.float32)
    b = rng.standard_normal((B, D)).astype(np.float32)
    print(kernel(a, b, None))


# revision 21
# speedup vs baseline: 1.5373x; 1.0610x over previous
"""Trainium2 Bass kernel for nn_ContrastiveLoss (SimCLR-style NT-Xent loss).

Reference computation:
    f = normalize(concat([z1, z2]))            # [2B, D] unit rows
    S = f @ f.T / T                            # [8192, 8192]
    loss = mean_i( logsumexp_j(S[i, :]) - S[i, pos_i] )

Symmetric sharding: S is symmetric, so each core computes only 5 of the
8 column-groups of its 1024-row block (groups 0..4 after rotating the 8
row-groups so the core's own rows are group 0).  The missing column
groups 5,6,7 of row-block b are the transposes of blocks computed by
cores b-3, b-2, b-1, and are recovered as COLUMN sums of the exp'd
blocks g=1..3 (a tiny fp8 DoubleRow ones-matmul per block), exchanged
between cores on the host during the final (cheap) reduction.  This cuts
matmul + exp work to 5/8 and HBM traffic to 10 MB/core.

Operand layout: rows are normalized in row-major bf16 (DVE sum-of-
squares + Quake rsqrt + scale), DMA-xbar transposed as native 2-byte
elements into [dp, db, col] (d = 128*db + dp), then cast to fp8e4.  A
DoubleRow matmul contraction pair (dp, t) maps to d = 256h + 128t + dp,
so BOTH operands slice straight out of the same [128, 4, 1024] fp8 tile
with far-strided (1024B) k-pairs and contiguous columns - the layout the
double-pumped weight/ifmap streams require (byte-interleaved pairs run
at 1 elem/cycle).  NOTE: tensor_tensor_reduce hangs TRN2 hardware (sim
is fine) - all mul-reduces must use affine_mul_reduce.

Per 128-row tile r and group g, the [128, 1024] psum block gets:
  g=0: diag extraction (eye mul-reduce, pre-exp), in-place exp with
       fused row-sum (accum_out).
  g=4: pos-pair extraction (same trick; pos offsets are +4B rows = group
       4 after rotation), in-place exp + row-sum.
  g=1..3: exp written as fp8e4 to SBUF (off-diagonal cosines are within
       +-0.25 whp, so exp(S/T) spans ~[e-4, e4] - inside fp8e4 range)
       + fused row-sum; pairs of row-tiles feed a [128, 2, 512] DR
       ones-matmul accumulating column sums in psum.

Host (f64) assembles denominators across cores:
  den[b] = rowsums_b - exp(diag_b/T) + e^{1/T} + sum_g colsums_{b-g}[g]
  loss   = mean(log(den) - pos/T)
The exact-diagonal substitution cancels the fp8 quantization noise of
the dominant e^{1/T} ~ 1.6e6 softmax term (the rest of a row sums to
~1e4), exactly as in the v1 kernel.  No logsumexp max-subtraction is
needed: sum_j exp() <= ~2e10 fits fp32.
"""

import os
import sys

try:
    import concourse.bass  # noqa: F401
except ImportError:
    for _p in ("/root/.axon_site/_ro/trn_rl_repo", "/opt/trn_rl_repo"):
        if _p not in sys.path and os.path.isdir(_p):
            sys.path.insert(0, _p)

import numpy as np

B = 4096
D = 512
T = 0.07
P = 128
NCORES = 8
R = (2 * B) // NCORES  # 1024 rows per block
G = 8                  # total row/col blocks
NG = 5                 # column groups computed per core (symmetry)
GT = R // P            # 8 row tiles per block
H = 2                  # DR contraction halves (256 each)
DB = D // P            # 4 d-blocks of 128

_NC = None


def _build():
    from contextlib import ExitStack

    import concourse.bacc as bacc
    import concourse.tile as tile
    from concourse import mybir
    from concourse.tile import add_dep_helper

    f32 = mybir.dt.float32
    bf16 = mybir.dt.bfloat16
    f8 = mybir.dt.float8e4
    i32 = mybir.dt.int32
    AFT = mybir.ActivationFunctionType
    EXPF = AFT.Exp
    MUL = mybir.AluOpType.mult
    ADD = mybir.AluOpType.add
    SUB = mybir.AluOpType.subtract
    SHR = mybir.AluOpType.logical_shift_right
    DR = mybir.MatmulPerfMode.DoubleRow

    nc = bacc.Bacc(
        "TRN2", target_bir_lowering=False, debug=False, num_devices=NCORES
    )
    fg = [
        nc.dram_tensor(f"f{k}", [R, D], f32, kind="ExternalInput")
        for k in range(NG)
    ]
    eye = nc.dram_tensor("eye", [P, P], f32, kind="ExternalInput")
    sums_out = nc.dram_tensor("sums", [P, NG * GT], f32, kind="ExternalOutput")
    diag_out = nc.dram_tensor("diag", [P, GT], f32, kind="ExternalOutput")
    pos_out = nc.dram_tensor("pos", [P, GT], f32, kind="ExternalOutput")
    csum_out = nc.dram_tensor("csum", [1, 3 * R], f32, kind="ExternalOutput")

    with ExitStack() as ctx:
        tc = ctx.enter_context(tile.TileContext(nc))
        smalls = ctx.enter_context(tc.tile_pool(name="smalls", bufs=1))
        dumps = ctx.enter_context(tc.tile_pool(name="dumps", bufs=4))
        stats = ctx.enter_context(tc.tile_pool(name="stats", bufs=3))
        zbpool = ctx.enter_context(tc.tile_pool(name="zbpool", bufs=3))
        tbpool = ctx.enter_context(tc.tile_pool(name="tbpool", bufs=2))
        f8pool = ctx.enter_context(tc.tile_pool(name="f8pool", bufs=1))
        e8pool = ctx.enter_context(tc.tile_pool(name="e8pool", bufs=2))
        scrpool = ctx.enter_context(tc.tile_pool(name="scrpool", bufs=2))
        psum = ctx.enter_context(tc.tile_pool(name="psum", bufs=3, space="PSUM"))
        cspool = ctx.enter_context(tc.tile_pool(name="cspool", bufs=1, space="PSUM"))

        sums_sb = smalls.tile([P, NG * GT], f32, tag="sums_sb")
        diag_sb = smalls.tile([P, GT], f32, tag="diag_sb")
        pos_sb = smalls.tile([P, GT], f32, tag="pos_sb")
        csum_sb = smalls.tile([1, 3 * R], f32, tag="csum_sb")
        eye_sb = smalls.tile([P, P], f32, tag="eye_sb")
        nc.sync.dma_start(out=eye_sb[:], in_=eye[:, :])
        magic = smalls.tile([P, GT], i32, tag="magic")
        nc.vector.memset(magic[:], 0x5F3759DF)
        # DR stationary all-ones [128, 2, 1] with 16B pair stride.
        ones8 = smalls.tile([P, 2, 16], f8, tag="ones8")
        nc.vector.memset(ones8[:], 1.0)

        def mulsum(in0, in1, accum_col):
            # accum_col[p] = sum_x in0[p,x]*in1[p,x]; main out is a
            # throwaway broadcast AP.  (tensor_tensor_reduce would do the
            # same in one standard op but hangs TRN2 hardware.)
            dummy = dumps.tile([P, 1], f32, tag="dummy")
            nc.vector.affine_mul_reduce(
                out=dummy.broadcast_to(in0.shape),
                accum_out=accum_col,
                in0=in0,
                in1=in1,
                scale=1.0,
                bias=0.0,
            )

        def rsqrt(invn_dst, ssq):
            # 1/max(sqrt(s), eps) == min(rsqrt(s), 1e12); Quake bit-trick
            # + 2 Newton iterations, all on DVE.
            n = ssq.shape[1]
            h = stats.tile([P, n], i32, tag="h")
            nc.vector.tensor_scalar(h[:], ssq.bitcast(i32), 1, None, op0=SHR)
            y = stats.tile([P, n], f32, tag="y")
            nc.vector.tensor_tensor(y[:].bitcast(i32), magic[:, :n], h[:], op=SUB)
            a = stats.tile([P, n], f32, tag="a")
            for _ in range(2):
                nc.vector.tensor_mul(a[:], y[:], y[:])
                nc.vector.tensor_mul(a[:], a[:], ssq)
                nc.vector.tensor_scalar(a[:], a[:], -0.5, 1.5, op0=MUL, op1=ADD)
                nc.vector.tensor_mul(y[:], y[:], a[:])
            nc.vector.tensor_scalar_min(invn_dst, y[:], 1.0e12)

        load_insts = []
        zbs = {}

        def load_group(g):
            # Pacing: one cast-DMA only sustains ~150 GB/s, so run two
            # chunks in parallel (~300 GB/s, near the 358 HBM cap) and
            # chain chunk n behind chunk n-2 to keep arrival order.
            zb = zbpool.tile([P, GT, D], bf16, tag="zb")
            for s in range(2):
                ld = nc.gpsimd.dma_start(
                    out=zb[:, s * 4 : (s + 1) * 4, :],
                    in_=fg[g][s * 4 * P : (s + 1) * 4 * P, :].rearrange(
                        "(a p) d -> p a d", p=P
                    ),
                )
                n = len(load_insts)
                if n >= 2:
                    add_dep_helper(
                        ld.ins, load_insts[n - 2].ins, reason="pace loads"
                    )
                load_insts.append(ld)
            zbs[g] = zb

        ft8s = {}

        ssqs = {}

        def prep_ssq(g, a):
            if g not in ssqs:
                ssqs[g] = stats.tile(
                    [P, GT], f32, tag=f"ssq{g % 2}", name=f"ssq_{g}"
                )
            mulsum(zbs[g][:, a, :], zbs[g][:, a, :], ssqs[g][:, a : a + 1])

        def prep_finish(g, two_queues=False):
            # rsqrt + row scale + transpose + fp8 cast for group g (the
            # 8 ssq mul-reduces were already emitted via prep_ssq).
            zb = zbs.pop(g)
            ssq = ssqs.pop(g)
            invn = stats.tile([P, GT], f32, tag="invn")
            rsqrt(invn[:], ssq[:])
            # Native 2-byte xbar transposes (one per row-tile a, keeping
            # both APs within the 2D-in/3D-out transpose constraint):
            # tb[dp, db, a*128+j] = zb[j, a, 128*db+dp], i.e. F^T with
            # d = 128*db + dp on the partition axis and columns ordered
            # like rows (a*128+j).  Each transpose is emitted right after
            # its row-tile's scale so it starts as early as possible.
            # During startup (ACT idle) they are split over both HWDGE
            # queues (SP + ACT) to halve the serial 8 x ~1.27us queue
            # time; in steady state ACT runs exps, so everything stays on
            # the SP queue.  The fp8 cast runs per column half so the
            # first matmuls only wait on 4 transposes.
            tb = tbpool.tile([P, DB, R], bf16, tag="tb")
            ft8 = f8pool.tile([P, DB, R], f8, tag=f"ft8_{g}", name=f"ft8_{g}")
            for half in range(2):
                for a in range(4 * half, 4 * half + 4):
                    nc.vector.tensor_scalar_mul(
                        zb[:, a, :], zb[:, a, :], invn[:, a : a + 1]
                    )
                    q = nc.scalar if (two_queues and a % 2 == 1) else nc.sync
                    q.dma_start(
                        out=tb[:, :, a * P : (a + 1) * P],
                        in_=zb[:, a, :],
                        transpose=True,
                    )
                sl = slice(half * 512, half * 512 + 512)
                nc.vector.tensor_copy(ft8[:, :, sl], tb[:, :, sl])
            ft8s[g] = ft8

        def prep_group(g, two_queues=False):
            for a in range(GT):
                prep_ssq(g, a)
            prep_finish(g, two_queues)

        def sim_phase(g, prep_g=None):
            # prep_g: group whose ssq mul-reduces are drip-fed one per
            # row-tile into the DVE queue (between this phase's psum
            # extractions), with the rsqrt/scale/transpose/cast tail
            # emitted after the phase - ready one full phase before use.
            ft8g = ft8s[g]
            ft80 = ft8s[0]
            cs = None
            if g in (1, 2, 3):
                cs = cspool.tile([P, R], f32, tag="cs")
            e8 = None
            for r in range(GT):
                ps = psum.tile([P, R], f32, tag="ps")
                for h in range(H):
                    lhsT = ft80[:, 2 * h : 2 * h + 2, r * P : (r + 1) * P]
                    for ns in range(2):
                        nc.tensor.matmul(
                            ps[:, ns * 512 : (ns + 1) * 512],
                            lhsT,
                            ft8g[:, 2 * h : 2 * h + 2, ns * 512 : (ns + 1) * 512],
                            start=(h == 0),
                            stop=(h == H - 1),
                            perf_mode=DR,
                        )
                # Raw-cosine extraction reads PSUM in parallel with exp
                # (the exp writes SBUF scratch, not in-place, so there is
                # no write-after-read ordering between them).
                if g == 0:
                    mulsum(ps[:, r * P : (r + 1) * P], eye_sb[:], diag_sb[:, r : r + 1])
                if g == 4:
                    mulsum(ps[:, r * P : (r + 1) * P], eye_sb[:], pos_sb[:, r : r + 1])
                acc = sums_sb[:, g * GT + r : g * GT + r + 1]
                if g in (1, 2, 3):
                    if r % 2 == 0:
                        e8 = e8pool.tile([P, 2, R], f8, tag="e8")
                    nc.scalar.activation(
                        e8[:, r % 2, :], ps[:], EXPF, scale=1.0 / T, accum_out=acc
                    )
                    if r % 2 == 1:
                        pr = r // 2
                        for ns in range(2):
                            nc.tensor.matmul(
                                cs[0:1, ns * 512 : (ns + 1) * 512],
                                ones8[:, :, 0:1],
                                e8[:, :, ns * 512 : (ns + 1) * 512],
                                start=(pr == 0),
                                stop=(pr == GT // 2 - 1),
                                perf_mode=DR,
                            )
                else:
                    scr = scrpool.tile([P, R], bf16, tag="scr")
                    nc.scalar.activation(
                        scr[:], ps[:], EXPF, scale=1.0 / T, accum_out=acc
                    )
                if prep_g is not None:
                    prep_ssq(prep_g, r)
            if g in (1, 2, 3):
                nc.vector.tensor_copy(
                    csum_sb[0:1, (g - 1) * R : g * R], cs[0:1, :]
                )
            if prep_g is not None:
                prep_finish(prep_g)

        # Startup: groups 0 and 1 loaded+prepped before phase 0 (using
        # both HWDGE queues for the transposes while ACT is idle); later
        # groups' loads start immediately and their DVE prep is drip-fed
        # through the preceding phases.
        load_group(0)
        load_group(1)
        prep_group(0, two_queues=True)
        prep_group(1, two_queues=True)
        for g in range(NG):
            if g + 2 < NG:
                load_group(g + 2)
            sim_phase(g, prep_g=g + 2 if g + 2 < NG else None)

        nc.sync.dma_start(out=sums_out[:], in_=sums_sb[:])
        nc.sync.dma_start(out=diag_out[:], in_=diag_sb[:])
        nc.sync.dma_start(out=pos_out[:], in_=pos_sb[:])
        nc.sync.dma_start(out=csum_out[:, :], in_=csum_sb[0:1, :])

    nc.compile()
    return nc


def _get_nc():
    global _NC
    if _NC is None:
        _NC = _build()
    return _NC


def run(z1, z2, trace=False):
    """Run the SPMD kernel; returns (loss, BassKernelResults)."""
    from concourse.bass_utils import run_bass_kernel_spmd

    z1 = np.ascontiguousarray(z1, dtype=np.float32)
    z2 = np.ascontiguousarray(z2, dtype=np.float32)
    F = np.concatenate([z1, z2], axis=0)  # [8192, 512]
    eye_np = np.eye(P, dtype=np.float32)
    in_maps = []
    for c in range(NCORES):
        m = {"eye": eye_np}
        for k in range(NG):
            blk = (c + k) % G
            m[f"f{k}"] = F[blk * R : (blk + 1) * R]
        in_maps.append(m)
    res = run_bass_kernel_spmd(
        _get_nc(), in_maps, core_ids=list(range(NCORES)), trace=trace
    )
    e_diag_true = np.exp(1.0 / T)
    # Per-core row-major [1024] views; row i = rt*128 + p.
    RS, DG, PS, CSa = [], [], [], []
    for r in res.results:
        sums = r["sums"].astype(np.float64)  # [P, NG*GT]
        RS.append(sums.reshape(P, NG, GT).sum(axis=1).T.reshape(R))
        DG.append(r["diag"].astype(np.float64).T.reshape(R))
        PS.append(r["pos"].astype(np.float64).T.reshape(R))
        CSa.append(r["csum"].astype(np.float64).reshape(3, R))  # row g-1
    total = 0.0
    for b in range(G):
        den = RS[b] - np.exp(DG[b] / T) + e_diag_true
        for g in (1, 2, 3):
            den = den + CSa[(b - g) % G][g - 1]
        total += (np.log(den) - PS[b] / T).sum()
    loss = total / (2.0 * B)
    return np.float32(loss), res


def kernel(z1, z2, labels=None, **_ignored):
    loss, _ = run(z1, z2, trace=False)
    return np.asarray(loss, dtype=np.float32)


if __name__ == "__main__":
    rng = np.random.default_rng(0)
    a = rng.standard_normal((B, D)).astype(np.float32)
    b = rng.standard_normal((B, D)).astype(np.float32)
    print(kernel(a, b, None))
